# revision 6
# baseline (speedup 1.0000x reference)
"""GRU layer kernel for Trainium2 (8 NeuronCores, data-parallel over batch).

Problem: B=64, T=1024, I=H=512 GRU (cuDNN-style gates with z roles swapped):
    r = sigmoid(x W_ir^T + h W_hr^T + b_hr)
    z = sigmoid(x W_iz^T + h W_hz^T + b_hz)
    n = tanh  (x W_in^T + (r*h) W_hn^T + b_hn)
    h' = (1-z)*h + z*n

Design (per core, B_loc=8):
  * Everything lives in [feature-on-partitions, batch-on-free] layout.
  * Input projections are hoisted: per time-chunk of 64 steps, 3 big GEMMs
    compute pre-activations xr/xz/xn for 512 tokens at once (fp16 operands,
    fp32 PSUM accumulate), biases folded in during the PSUM->SBUF copy.
  * The sequential recurrence runs 48 small matmuls per step (3 gates x
    4 K-tiles x 4 M-tiles, stationary fp16 weights, 8-column moving h).
    fp16 stationary weights get hardware fast-weight-load; h is kept as an
    fp32 master copy (elementwise math) plus an fp16 streaming copy.
  * Outputs accumulate per chunk in [H, token] layout, are PE-transposed to
    [token, H] and DMA'd out densely.
"""

import numpy as np

import concourse.bass as bass
import concourse.mybir as mybir
import concourse.tile as tile
from concourse import bacc
from concourse.bass_utils import run_bass_kernel_spmd
from concourse.masks import make_identity

B, T, I, H = 64, 1024, 512, 512
NCORES = 8
BL = B // NCORES  # 8 batch elements per core
TC = 64           # time steps per chunk
KT = I // 128     # 4 contraction tiles
MT = H // 128     # 4 output tiles per gate
GT = 3            # gates r, z, n
F32 = mybir.dt.float32
F16 = mybir.dt.float16

W_NAMES = ["W_ir", "W_iz", "W_in", "W_hr", "W_hz", "W_hn"]
B_NAMES = ["b_hr", "b_hz", "b_hn"]

ActF = mybir.ActivationFunctionType


def build(nc_T=T, tc_sz=TC):
    nchunk = nc_T // tc_sz
    ntok = BL * tc_sz        # tokens per chunk (512 at TC=64)
    ntt = ntok // 128        # 128-token tiles per chunk

    nc = bacc.Bacc("TRN2", target_bir_lowering=False, debug=False,
                   num_devices=NCORES)
    x_d = nc.dram_tensor("x", [BL, nc_T, I], F32, kind="ExternalInput").ap()
    h0_d = nc.dram_tensor("h_0", [BL, H], F32, kind="ExternalInput").ap()
    w_d = {n: nc.dram_tensor(n, [H, I], F32, kind="ExternalInput").ap()
           for n in W_NAMES}
    b_d = {n: nc.dram_tensor(n, [H], F32, kind="ExternalInput").ap()
           for n in B_NAMES}
    y_d = nc.dram_tensor("y", [BL, nc_T, H], F32, kind="ExternalOutput").ap()

    with tile.TileContext(nc) as tcx, \
            tcx.tile_pool(name="consts", bufs=1) as consts:
        ident = consts.tile([128, 128], F32, name="ident")
        make_identity(nc, ident)

        # Transposed fp16 weights, one tile per K-tile, gates concatenated in
        # the free dim: wiT[k][:, g*H + j] = W_i<g>[j, k*128 + p].
        wiT = [consts.tile([128, GT * H], F16, name=f"wiT{k}") for k in range(KT)]
        whT = [consts.tile([128, GT * H], F16, name=f"whT{k}") for k in range(KT)]
        b_sb = consts.tile([128, GT * MT], F32, name="b_sb")
        for g, n in enumerate(B_NAMES):
            nc.sync.dma_start(b_sb[:, g * MT:(g + 1) * MT],
                              b_d[n].rearrange("(m p) -> p m", p=128))

        with (
            tcx.tile_pool(name="wraw", bufs=2) as wraw_pool,
            tcx.tile_pool(name="tp_ps", bufs=2, space="PSUM") as tp_pool,
            tcx.tile_pool(name="gemm_ps", bufs=2, space="PSUM") as gemm_pool,
            tcx.tile_pool(name="prz_ps", bufs=2, space="PSUM") as prz_pool,
            tcx.tile_pool(name="pn_ps", bufs=2, space="PSUM") as pn_pool,
            tcx.tile_pool(name="xsb", bufs=3) as xsb_pool,
            tcx.tile_pool(name="xt16", bufs=2) as xt_pool,
            tcx.tile_pool(name="xrt", bufs=2) as xrt_pool,
            tcx.tile_pool(name="outb", bufs=2) as outb_pool,
            tcx.tile_pool(name="stage", bufs=3) as stage_pool,
            tcx.tile_pool(name="step", bufs=3) as step_pool,
        ):
            # ---- one-time: transpose weights into [i, j] fp16 layout ----
            for wi, n in enumerate(W_NAMES):
                dest = wiT if wi < 3 else whT
                g = wi % 3
                for jt in range(4):
                    wr = wraw_pool.tile([128, I], F32, tag="wraw")
                    nc.sync.dma_start(wr, w_d[n][jt * 128:(jt + 1) * 128, :])
                    for it in range(KT):
                        tp = tp_pool.tile([128, 128], F32, tag="tp")
                        nc.tensor.transpose(tp, wr[:, it * 128:(it + 1) * 128],
                                            ident)
                        nc.vector.tensor_copy(
                            dest[it][:, g * H + jt * 128: g * H + (jt + 1) * 128],
                            tp)

            # ---- one-time: h0 -> transposed fp32 + fp16 ping-pong bufs ----
            hf = [consts.tile([128, MT * BL], F32, name=f"hf{i}") for i in range(2)]
            h16 = [consts.tile([128, MT * BL], F16, name=f"h16_{i}") for i in range(2)]
            h0_sb = consts.tile([BL, H], F32, name="h0_sb")
            nc.sync.dma_start(h0_sb, h0_d)
            for k in range(KT):
                tp = tp_pool.tile([128, BL], F32, tag="tp")
                nc.tensor.transpose(tp, h0_sb[:, k * 128:(k + 1) * 128],
                                    ident[0:BL, 0:BL])
                nc.vector.tensor_copy(hf[0][:, k * BL:(k + 1) * BL], tp)
            nc.scalar.copy(h16[0], hf[0])

            for c in range(nchunk):
                t0 = c * tc_sz
                # ---- input projections for this chunk ----
                xt = xt_pool.tile([128, KT * ntok], F16, tag="xt")
                for q in range(ntt):
                    b0 = q * (128 // tc_sz)
                    xs = xsb_pool.tile([128, I], F32, tag="xs")
                    for bb in range(128 // tc_sz):
                        nc.sync.dma_start(xs[bb * tc_sz:(bb + 1) * tc_sz, :],
                                          x_d[b0 + bb, t0:t0 + tc_sz, :])
                    for k in range(KT):
                        tp = tp_pool.tile([128, 128], F32, tag="tp")
                        nc.tensor.transpose(tp, xs[:, k * 128:(k + 1) * 128],
                                            ident)
                        nc.vector.tensor_copy(
                            xt[:, k * ntok + q * 128: k * ntok + (q + 1) * 128],
                            tp)
                xrt = xrt_pool.tile([128, GT * MT * ntok], F32, tag="xrt")
                for g in range(GT):
                    for m in range(MT):
                        gp = gemm_pool.tile([128, ntok], F32, tag="gemm")
                        for k in range(KT):
                            nc.tensor.matmul(
                                gp,
                                wiT[k][:, g * H + m * 128: g * H + (m + 1) * 128],
                                xt[:, k * ntok:(k + 1) * ntok],
                                start=(k == 0), stop=(k == KT - 1))
                        nc.scalar.add(
                            xrt[:, (g * MT + m) * ntok:(g * MT + m + 1) * ntok],
                            gp, add=b_sb[:, g * MT + m: g * MT + m + 1])

                # chunk pre-acts viewed as [p, gate, mtile, batch, dt]
                xv = xrt.rearrange("p (g m b t) -> p g m b t", g=GT, m=MT, b=BL)
                out_buf = outb_pool.tile([128, MT * ntok], F32, tag="outb")
                ov = out_buf.rearrange("p (m b t) -> p m b t", m=MT, b=BL)

                # ---- sequential recurrence over the chunk ----
                for dt in range(tc_sz):
                    t = t0 + dt
                    p, pn = t % 2, (t + 1) % 2
                    prz = prz_pool.tile([128, 2 * MT * BL], F32, tag="prz")
                    for g in range(2):  # r, z
                        for m in range(MT):
                            o0 = g * MT * BL + m * BL
                            for k in range(KT):
                                nc.tensor.matmul(
                                    prz[:, o0:o0 + BL],
                                    whT[k][:, g * H + m * 128: g * H + (m + 1) * 128],
                                    h16[p][:, k * BL:(k + 1) * BL],
                                    start=(k == 0), stop=(k == KT - 1))
                    rz_sb = step_pool.tile([128, 2 * MT * BL], F32, tag="rz_sb")
                    nc.vector.tensor_add(rz_sb[:, 0:MT * BL],
                                         prz[:, 0:MT * BL], xv[:, 0, :, :, dt])
                    nc.vector.tensor_add(rz_sb[:, MT * BL:],
                                         prz[:, MT * BL:], xv[:, 1, :, :, dt])
                    rz_act = step_pool.tile([128, 2 * MT * BL], F32, tag="rz_act")
                    nc.scalar.activation(rz_act, rz_sb, ActF.Sigmoid)
                    rh = step_pool.tile([128, MT * BL], F16, tag="rh")
                    nc.vector.tensor_mul(rh, rz_act[:, 0:MT * BL], hf[p])
                    pn_ps = pn_pool.tile([128, MT * BL], F32, tag="pn")
                    for m in range(MT):
                        for k in range(KT):
                            nc.tensor.matmul(
                                pn_ps[:, m * BL:(m + 1) * BL],
                                whT[k][:, 2 * H + m * 128: 2 * H + (m + 1) * 128],
                                rh[:, k * BL:(k + 1) * BL],
                                start=(k == 0), stop=(k == KT - 1))
                    n_sb = step_pool.tile([128, MT * BL], F32, tag="n_sb")
                    nc.vector.tensor_add(n_sb, pn_ps, xv[:, 2, :, :, dt])
                    n_act = step_pool.tile([128, MT * BL], F32, tag="n_act")
                    nc.scalar.activation(n_act, n_sb, ActF.Tanh)
                    d_sb = step_pool.tile([128, MT * BL], F32, tag="d_sb")
                    nc.vector.tensor_sub(d_sb, n_act, hf[p])
                    v_sb = step_pool.tile([128, MT * BL], F32, tag="v_sb")
                    nc.vector.tensor_mul(v_sb, rz_act[:, MT * BL:], d_sb)
                    nc.vector.tensor_add(hf[pn], hf[p], v_sb)
                    nc.scalar.copy(h16[pn], hf[pn])
                    nc.scalar.copy(ov[:, :, :, dt], hf[pn])

                # ---- transpose chunk outputs to [token, H] and store ----
                for q in range(ntt):
                    b0 = q * (128 // tc_sz)
                    st = stage_pool.tile([128, H], F32, tag="stage")
                    for m in range(MT):
                        tp = tp_pool.tile([128, 128], F32, tag="tp")
                        nc.tensor.transpose(
                            tp, out_buf[:, m * ntok + q * 128: m * ntok + (q + 1) * 128],
                            ident)
                        nc.vector.tensor_copy(st[:, m * 128:(m + 1) * 128], tp)
                    for bb in range(128 // tc_sz):
                        nc.sync.dma_start(y_d[b0 + bb, t0:t0 + tc_sz, :],
                                          st[bb * tc_sz:(bb + 1) * tc_sz, :])

    nc.compile()
    return nc


_CACHE = {}


def _get_nc():
    if "nc" not in _CACHE:
        _CACHE["nc"] = build()
    return _CACHE["nc"]


def kernel(x, h_0, W_ir, W_iz, W_in, W_hr, W_hz, W_hn, b_hr, b_hz, b_hn):
    args = dict(x=x, h_0=h_0, W_ir=W_ir, W_iz=W_iz, W_in=W_in, W_hr=W_hr,
                W_hz=W_hz, W_hn=W_hn, b_hr=b_hr, b_hz=b_hz, b_hn=b_hn)
    args = {k: np.ascontiguousarray(np.asarray(v, dtype=np.float32))
            for k, v in args.items()}
    nc = _get_nc()
    in_maps = []
    for c in range(NCORES):
        m = {n: args[n] for n in W_NAMES + B_NAMES}
        m["x"] = args["x"][c * BL:(c + 1) * BL]
        m["h_0"] = args["h_0"][c * BL:(c + 1) * BL]
        in_maps.append(m)
    res = run_bass_kernel_spmd(nc, in_maps, core_ids=list(range(NCORES)))
    outputs = np.concatenate([res.results[c]["y"] for c in range(NCORES)],
                             axis=0)
    return outputs, outputs[:, -1].copy()


# revision 11
# speedup vs baseline: 1.0095x; 1.0095x over previous
"""GRU layer kernel for Trainium2 (8 NeuronCores, data-parallel over batch).

Problem: B=64, T=1024, I=H=512 GRU (cuDNN-style gates with z roles swapped):
    r = sigmoid(x W_ir^T + h W_hr^T + b_hr)
    z = sigmoid(x W_iz^T + h W_hz^T + b_hz)
    n = tanh  (x W_in^T + (r*h) W_hn^T + b_hn)
    h' = (1-z)*h + z*n

Design (per core, B_loc=8):
  * Everything lives in [feature-on-partitions, batch-on-free] layout.
  * Input projections are hoisted: per time-chunk of 64 steps, 3 big GEMMs
    compute pre-activations xr/xz/xn for 512 tokens at once (fp16 operands,
    fp32 PSUM accumulate), biases folded in during the PSUM->SBUF copy.
  * The sequential recurrence runs 48 small matmuls per step (3 gates x
    4 K-tiles x 4 M-tiles, stationary fp16 weights, 8-column moving h).
    fp16 stationary weights get hardware fast-weight-load; h is kept as an
    fp32 master copy (elementwise math) plus an fp16 streaming copy.
  * Outputs accumulate per chunk in [H, token] layout, are PE-transposed to
    [token, H] and DMA'd out densely.
"""

import numpy as np

import concourse.bass as bass
import concourse.mybir as mybir
import concourse.tile as tile
from concourse import bacc
from concourse.bass_utils import run_bass_kernel_spmd
from concourse.masks import make_identity

B, T, I, H = 64, 1024, 512, 512
NCORES = 8
BL = B // NCORES  # 8 batch elements per core
TC = 64           # time steps per chunk
KT = I // 128     # 4 contraction tiles
MT = H // 128     # 4 output tiles per gate
GT = 3            # gates r, z, n
F32 = mybir.dt.float32
F16 = mybir.dt.float16

W_NAMES = ["W_ir", "W_iz", "W_in", "W_hr", "W_hz", "W_hn"]
B_NAMES = ["b_hr", "b_hz", "b_hn"]

ActF = mybir.ActivationFunctionType


def build(nc_T=T, tc_sz=TC):
    nchunk = nc_T // tc_sz
    ntok = BL * tc_sz        # tokens per chunk (512 at TC=64)
    ntt = ntok // 128        # 128-token tiles per chunk

    nc = bacc.Bacc("TRN2", target_bir_lowering=False, debug=False,
                   num_devices=NCORES)
    x_d = nc.dram_tensor("x", [BL, nc_T, I], F32, kind="ExternalInput").ap()
    h0_d = nc.dram_tensor("h_0", [BL, H], F32, kind="ExternalInput").ap()
    w_d = {n: nc.dram_tensor(n, [H, I], F32, kind="ExternalInput").ap()
           for n in W_NAMES}
    b_d = {n: nc.dram_tensor(n, [H], F32, kind="ExternalInput").ap()
           for n in B_NAMES}
    y_d = nc.dram_tensor("y", [BL, nc_T, H], F32, kind="ExternalOutput").ap()

    with tile.TileContext(nc) as tcx, \
            tcx.tile_pool(name="consts", bufs=1) as consts:
        ident = consts.tile([128, 128], F32, name="ident")
        make_identity(nc, ident)
        ident16 = consts.tile([128, 128], F16, name="ident16")
        nc.vector.tensor_copy(ident16, ident)

        # Transposed fp16 weights, one tile per K-tile, gates concatenated in
        # the free dim: wiT[k][:, g*H + j] = W_i<g>[j, k*128 + p].
        wiT = [consts.tile([128, GT * H], F16, name=f"wiT{k}") for k in range(KT)]
        whT = [consts.tile([128, GT * H], F16, name=f"whT{k}") for k in range(KT)]
        b_sb = consts.tile([128, GT * MT], F32, name="b_sb")
        for g, n in enumerate(B_NAMES):
            nc.sync.dma_start(b_sb[:, g * MT:(g + 1) * MT],
                              b_d[n].rearrange("(m p) -> p m", p=128))

        with (
            tcx.tile_pool(name="wraw", bufs=2) as wraw_pool,
            tcx.tile_pool(name="tp_ps", bufs=2, space="PSUM") as tp_pool,
            tcx.tile_pool(name="gemm_ps", bufs=2, space="PSUM") as gemm_pool,
            tcx.tile_pool(name="prz_ps", bufs=2, space="PSUM") as prz_pool,
            tcx.tile_pool(name="pn_ps", bufs=2, space="PSUM") as pn_pool,
            tcx.tile_pool(name="xsb", bufs=3) as xsb_pool,
            tcx.tile_pool(name="xt16", bufs=2) as xt_pool,
            tcx.tile_pool(name="xrt", bufs=2) as xrt_pool,
            tcx.tile_pool(name="outb", bufs=2) as outb_pool,
            tcx.tile_pool(name="stage", bufs=3) as stage_pool,
            tcx.tile_pool(name="step", bufs=3) as step_pool,
        ):
            # ---- one-time: transpose weights into [i, j] fp16 layout ----
            for wi, n in enumerate(W_NAMES):
                dest = wiT if wi < 3 else whT
                g = wi % 3
                for jt in range(4):
                    wr = wraw_pool.tile([128, I], F32, tag="wraw")
                    nc.sync.dma_start(wr, w_d[n][jt * 128:(jt + 1) * 128, :])
                    for it in range(KT):
                        tp = tp_pool.tile([128, 128], F32, tag="tp")
                        nc.tensor.transpose(tp, wr[:, it * 128:(it + 1) * 128],
                                            ident)
                        nc.vector.tensor_copy(
                            dest[it][:, g * H + jt * 128: g * H + (jt + 1) * 128],
                            tp)

            # ---- one-time: h0 -> transposed fp32 + fp16 ping-pong bufs ----
            hf = [consts.tile([128, MT * BL], F32, name=f"hf{i}") for i in range(2)]
            h16 = [consts.tile([128, MT * BL], F16, name=f"h16_{i}") for i in range(2)]
            h0_sb = consts.tile([BL, H], F32, name="h0_sb")
            nc.sync.dma_start(h0_sb, h0_d)
            for k in range(KT):
                tp = tp_pool.tile([128, BL], F32, tag="tp")
                nc.tensor.transpose(tp, h0_sb[:, k * 128:(k + 1) * 128],
                                    ident[0:BL, 0:BL])
                nc.vector.tensor_copy(hf[0][:, k * BL:(k + 1) * BL], tp)
            nc.scalar.copy(h16[0], hf[0])

            for c in range(nchunk):
                t0 = c * tc_sz
                # ---- input projections for this chunk ----
                xt = xt_pool.tile([128, KT * ntok], F16, tag="xt")
                for q in range(ntt):
                    b0 = q * (128 // tc_sz)
                    xs = xsb_pool.tile([128, I], F32, tag="xs")
                    for bb in range(128 // tc_sz):
                        nc.sync.dma_start(xs[bb * tc_sz:(bb + 1) * tc_sz, :],
                                          x_d[b0 + bb, t0:t0 + tc_sz, :])
                    xs16 = xsb_pool.tile([128, I], F16, tag="xs16")
                    nc.vector.tensor_copy(xs16, xs)
                    for k in range(KT):
                        tp16 = tp_pool.tile([128, 128], F16, tag="tp")
                        nc.tensor.transpose(tp16, xs16[:, k * 128:(k + 1) * 128],
                                            ident16)
                        nc.vector.tensor_copy(
                            xt[:, k * ntok + q * 128: k * ntok + (q + 1) * 128],
                            tp16)
                xrt = xrt_pool.tile([128, GT * MT * ntok], F32, tag="xrt")
                for g in range(GT):
                    for m in range(MT):
                        gp = gemm_pool.tile([128, ntok], F32, tag="gemm")
                        for k in range(KT):
                            nc.tensor.matmul(
                                gp,
                                wiT[k][:, g * H + m * 128: g * H + (m + 1) * 128],
                                xt[:, k * ntok:(k + 1) * ntok],
                                start=(k == 0), stop=(k == KT - 1))
                        nc.scalar.add(
                            xrt[:, (g * MT + m) * ntok:(g * MT + m + 1) * ntok],
                            gp, add=b_sb[:, g * MT + m: g * MT + m + 1])

                # chunk pre-acts viewed as [p, gate, mtile, batch, dt]
                xv = xrt.rearrange("p (g m b t) -> p g m b t", g=GT, m=MT, b=BL)
                out_buf = outb_pool.tile([128, MT * ntok], F16, tag="outb")
                ov = out_buf.rearrange("p (m b t) -> p m b t", m=MT, b=BL)

                # ---- sequential recurrence over the chunk ----
                for dt in range(tc_sz):
                    t = t0 + dt
                    p, pn = t % 2, (t + 1) % 2
                    prz = prz_pool.tile([128, 2 * MT * BL], F32, tag="prz")
                    for g in range(2):  # r, z
                        for m in range(MT):
                            o0 = g * MT * BL + m * BL
                            for k in range(KT):
                                nc.tensor.matmul(
                                    prz[:, o0:o0 + BL],
                                    whT[k][:, g * H + m * 128: g * H + (m + 1) * 128],
                                    h16[p][:, k * BL:(k + 1) * BL],
                                    start=(k == 0), stop=(k == KT - 1))
                    rz_sb = step_pool.tile([128, 2 * MT * BL], F32, tag="rz_sb")
                    nc.vector.tensor_add(rz_sb[:, 0:MT * BL],
                                         prz[:, 0:MT * BL], xv[:, 0, :, :, dt])
                    nc.vector.tensor_add(rz_sb[:, MT * BL:],
                                         prz[:, MT * BL:], xv[:, 1, :, :, dt])
                    rz_act = step_pool.tile([128, 2 * MT * BL], F32, tag="rz_act")
                    nc.scalar.activation(rz_act, rz_sb, ActF.Sigmoid)
                    rh = step_pool.tile([128, MT * BL], F16, tag="rh")
                    nc.vector.tensor_mul(rh, rz_act[:, 0:MT * BL], hf[p])
                    pn_ps = pn_pool.tile([128, MT * BL], F32, tag="pn")
                    for m in range(MT):
                        for k in range(KT):
                            nc.tensor.matmul(
                                pn_ps[:, m * BL:(m + 1) * BL],
                                whT[k][:, 2 * H + m * 128: 2 * H + (m + 1) * 128],
                                rh[:, k * BL:(k + 1) * BL],
                                start=(k == 0), stop=(k == KT - 1))
                    n_sb = step_pool.tile([128, MT * BL], F32, tag="n_sb")
                    nc.vector.tensor_add(n_sb, pn_ps, xv[:, 2, :, :, dt])
                    n_act = step_pool.tile([128, MT * BL], F32, tag="n_act")
                    nc.scalar.activation(n_act, n_sb, ActF.Tanh)
                    d_sb = step_pool.tile([128, MT * BL], F32, tag="d_sb")
                    nc.vector.tensor_sub(d_sb, n_act, hf[p])
                    v_sb = step_pool.tile([128, MT * BL], F32, tag="v_sb")
                    nc.vector.tensor_mul(v_sb, rz_act[:, MT * BL:], d_sb)
                    nc.vector.tensor_add(hf[pn], hf[p], v_sb)
                    nc.scalar.copy(h16[pn], hf[pn])
                    nc.scalar.copy(ov[:, :, :, dt], hf[pn])

                # ---- transpose chunk outputs to [token, H] and store ----
                for q in range(ntt):
                    b0 = q * (128 // tc_sz)
                    st = stage_pool.tile([128, H], F32, tag="stage")
                    for m in range(MT):
                        tp16 = tp_pool.tile([128, 128], F16, tag="tp")
                        nc.tensor.transpose(
                            tp16, out_buf[:, m * ntok + q * 128: m * ntok + (q + 1) * 128],
                            ident16)
                        nc.vector.tensor_copy(st[:, m * 128:(m + 1) * 128], tp16)
                    for bb in range(128 // tc_sz):
                        nc.sync.dma_start(y_d[b0 + bb, t0:t0 + tc_sz, :],
                                          st[bb * tc_sz:(bb + 1) * tc_sz, :])

    nc.compile()
    return nc


_CACHE = {}


def _get_nc():
    if "nc" not in _CACHE:
        _CACHE["nc"] = build()
    return _CACHE["nc"]


def kernel(x, h_0, W_ir, W_iz, W_in, W_hr, W_hz, W_hn, b_hr, b_hz, b_hn):
    args = dict(x=x, h_0=h_0, W_ir=W_ir, W_iz=W_iz, W_in=W_in, W_hr=W_hr,
                W_hz=W_hz, W_hn=W_hn, b_hr=b_hr, b_hz=b_hz, b_hn=b_hn)
    args = {k: np.ascontiguousarray(np.asarray(v, dtype=np.float32))
            for k, v in args.items()}
    nc = _get_nc()
    in_maps = []
    for c in range(NCORES):
        m = {n: args[n] for n in W_NAMES + B_NAMES}
        m["x"] = args["x"][c * BL:(c + 1) * BL]
        m["h_0"] = args["h_0"][c * BL:(c + 1) * BL]
        in_maps.append(m)
    res = run_bass_kernel_spmd(nc, in_maps, core_ids=list(range(NCORES)))
    outputs = np.concatenate([res.results[c]["y"] for c in range(NCORES)],
                             axis=0)
    return outputs, outputs[:, -1].copy()


# revision 15
# speedup vs baseline: 1.0104x; 1.0008x over previous
"""GRU layer kernel for Trainium2 (8 NeuronCores, data-parallel over batch).

Problem: B=64, T=1024, I=H=512 GRU (cuDNN-style gates with z roles swapped):
    r = sigmoid(x W_ir^T + h W_hr^T + b_hr)
    z = sigmoid(x W_iz^T + h W_hz^T + b_hz)
    n = tanh  (x W_in^T + (r*h) W_hn^T + b_hn)
    h' = (1-z)*h + z*n

Design (per core, B_loc=8):
  * Everything lives in [feature-on-partitions, batch-on-free] layout.
  * Input projections are hoisted: per time-chunk of 64 steps, 3 big GEMMs
    compute pre-activations xr/xz/xn for 512 tokens at once (fp16 operands,
    fp32 PSUM accumulate), biases folded in during the PSUM->SBUF copy.
  * The sequential recurrence runs 48 small matmuls per step (3 gates x
    4 K-tiles x 4 M-tiles, stationary fp16 weights, 8-column moving h).
    Measured cost is LDWEIGHTS-bound (~105ns per 128x128 stationary tile);
    the per-step elementwise chain hides entirely under the weight loads
    via the PE's load-ahead window. h is kept as an fp32 master copy
    (elementwise math) plus an fp16 streaming copy.
  * Outputs accumulate per chunk in fp16 [H, token] layout, are
    PE-transposed (single-pass fp16) to [token, H] and DMA'd out densely.
"""

import numpy as np

import concourse.bass as bass
import concourse.mybir as mybir
import concourse.tile as tile
from concourse import bacc
from concourse.bass_utils import run_bass_kernel_spmd
from concourse.masks import make_identity

B, T, I, H = 64, 1024, 512, 512
NCORES = 8
BL = B // NCORES  # 8 batch elements per core
TC = 64           # time steps per chunk
KT = I // 128     # 4 contraction tiles
MT = H // 128     # 4 output tiles per gate
GT = 3            # gates r, z, n
F32 = mybir.dt.float32
F16 = mybir.dt.float16

W_NAMES = ["W_ir", "W_iz", "W_in", "W_hr", "W_hz", "W_hn"]
B_NAMES = ["b_hr", "b_hz", "b_hn"]

ActF = mybir.ActivationFunctionType


def build(nc_T=T, tc_sz=TC):
    nchunk = nc_T // tc_sz
    ntok = BL * tc_sz        # tokens per chunk (512 at TC=64)
    ntt = ntok // 128        # 128-token tiles per chunk

    nc = bacc.Bacc("TRN2", target_bir_lowering=False, debug=False,
                   num_devices=NCORES)
    x_d = nc.dram_tensor("x", [BL, nc_T, I], F32, kind="ExternalInput").ap()
    h0_d = nc.dram_tensor("h_0", [BL, H], F32, kind="ExternalInput").ap()
    w_d = {n: nc.dram_tensor(n, [H, I], F32, kind="ExternalInput").ap()
           for n in W_NAMES}
    b_d = {n: nc.dram_tensor(n, [H], F32, kind="ExternalInput").ap()
           for n in B_NAMES}
    y_d = nc.dram_tensor("y", [BL, nc_T, H], F32, kind="ExternalOutput").ap()

    with tile.TileContext(nc) as tcx, \
            tcx.tile_pool(name="consts", bufs=1) as consts:
        ident = consts.tile([128, 128], F32, name="ident")
        make_identity(nc, ident)
        ident16 = consts.tile([128, 128], F16, name="ident16")
        nc.vector.tensor_copy(ident16, ident)

        # Transposed fp16 weights, one tile per K-tile, gates concatenated in
        # the free dim: wiT[k][:, g*H + j] = W_i<g>[j, k*128 + p].
        wiT = [consts.tile([128, GT * H], F16, name=f"wiT{k}") for k in range(KT)]
        whT = [consts.tile([128, GT * H], F16, name=f"whT{k}") for k in range(KT)]
        b_sb = consts.tile([128, GT * MT], F32, name="b_sb")
        for g, n in enumerate(B_NAMES):
            nc.sync.dma_start(b_sb[:, g * MT:(g + 1) * MT],
                              b_d[n].rearrange("(m p) -> p m", p=128))

        with (
            tcx.tile_pool(name="wraw", bufs=2) as wraw_pool,
            tcx.tile_pool(name="tp_ps", bufs=2, space="PSUM") as tp_pool,
            tcx.tile_pool(name="gemm_ps", bufs=2, space="PSUM") as gemm_pool,
            tcx.tile_pool(name="prz_ps", bufs=2, space="PSUM") as prz_pool,
            tcx.tile_pool(name="pn_ps", bufs=2, space="PSUM") as pn_pool,
            tcx.tile_pool(name="xsb", bufs=3) as xsb_pool,
            tcx.tile_pool(name="xt16", bufs=2) as xt_pool,
            tcx.tile_pool(name="xrt", bufs=2) as xrt_pool,
            tcx.tile_pool(name="outb", bufs=2) as outb_pool,
            tcx.tile_pool(name="stage", bufs=3) as stage_pool,
            tcx.tile_pool(name="step", bufs=3) as step_pool,
        ):
            # ---- one-time: transpose weights into [i, j] fp16 layout ----
            for wi, n in enumerate(W_NAMES):
                dest = wiT if wi < 3 else whT
                g = wi % 3
                for jt in range(4):
                    wr = wraw_pool.tile([128, I], F32, tag="wraw")
                    nc.sync.dma_start(wr, w_d[n][jt * 128:(jt + 1) * 128, :])
                    wr16 = wraw_pool.tile([128, I], F16, tag="wraw16")
                    nc.vector.tensor_copy(wr16, wr)
                    for it in range(KT):
                        tp = tp_pool.tile([128, 128], F16, tag="tp")
                        nc.tensor.transpose(tp, wr16[:, it * 128:(it + 1) * 128],
                                            ident16)
                        nc.vector.tensor_copy(
                            dest[it][:, g * H + jt * 128: g * H + (jt + 1) * 128],
                            tp)

            # ---- one-time: h0 -> transposed fp32 + fp16 ping-pong bufs ----
            hf = [consts.tile([128, MT * BL], F32, name=f"hf{i}") for i in range(2)]
            h16 = [consts.tile([128, MT * BL], F16, name=f"h16_{i}") for i in range(2)]
            h0_sb = consts.tile([BL, H], F32, name="h0_sb")
            nc.sync.dma_start(h0_sb, h0_d)
            for k in range(KT):
                tp = tp_pool.tile([128, BL], F32, tag="tp")
                nc.tensor.transpose(tp, h0_sb[:, k * 128:(k + 1) * 128],
                                    ident[0:BL, 0:BL])
                nc.vector.tensor_copy(hf[0][:, k * BL:(k + 1) * BL], tp)
            nc.scalar.copy(h16[0], hf[0])

            for c in range(nchunk):
                t0 = c * tc_sz
                # ---- input projections for this chunk ----
                xt = xt_pool.tile([128, KT * ntok], F16, tag="xt")
                for q in range(ntt):
                    b0 = q * (128 // tc_sz)
                    xs = xsb_pool.tile([128, I], F32, tag="xs")
                    for bb in range(128 // tc_sz):
                        nc.sync.dma_start(xs[bb * tc_sz:(bb + 1) * tc_sz, :],
                                          x_d[b0 + bb, t0:t0 + tc_sz, :])
                    xs16 = xsb_pool.tile([128, I], F16, tag="xs16")
                    nc.vector.tensor_copy(xs16, xs)
                    for k in range(KT):
                        tp16 = tp_pool.tile([128, 128], F16, tag="tp")
                        nc.tensor.transpose(tp16, xs16[:, k * 128:(k + 1) * 128],
                                            ident16)
                        nc.vector.tensor_copy(
                            xt[:, k * ntok + q * 128: k * ntok + (q + 1) * 128],
                            tp16)
                xrt = xrt_pool.tile([128, GT * MT * ntok], F32, tag="xrt")
                for g in range(GT):
                    for m in range(MT):
                        gp = gemm_pool.tile([128, ntok], F32, tag="gemm")
                        for k in range(KT):
                            nc.tensor.matmul(
                                gp,
                                wiT[k][:, g * H + m * 128: g * H + (m + 1) * 128],
                                xt[:, k * ntok:(k + 1) * ntok],
                                start=(k == 0), stop=(k == KT - 1))
                        nc.scalar.add(
                            xrt[:, (g * MT + m) * ntok:(g * MT + m + 1) * ntok],
                            gp, add=b_sb[:, g * MT + m: g * MT + m + 1])

                # chunk pre-acts viewed as [p, gate, mtile, batch, dt]
                xv = xrt.rearrange("p (g m b t) -> p g m b t", g=GT, m=MT, b=BL)
                out_buf = outb_pool.tile([128, MT * ntok], F16, tag="outb")
                ov = out_buf.rearrange("p (m b t) -> p m b t", m=MT, b=BL)

                # ---- sequential recurrence over the chunk ----
                for dt in range(tc_sz):
                    t = t0 + dt
                    p, pn = t % 2, (t + 1) % 2
                    prz = prz_pool.tile([128, 2 * MT * BL], F32, tag="prz")
                    for g in range(2):  # r, z
                        for m in range(MT):
                            o0 = g * MT * BL + m * BL
                            for k in range(KT):
                                nc.tensor.matmul(
                                    prz[:, o0:o0 + BL],
                                    whT[k][:, g * H + m * 128: g * H + (m + 1) * 128],
                                    h16[p][:, k * BL:(k + 1) * BL],
                                    start=(k == 0), stop=(k == KT - 1))
                    rz_sb = step_pool.tile([128, 2 * MT * BL], F32, tag="rz_sb")
                    nc.vector.tensor_add(rz_sb[:, 0:MT * BL],
                                         prz[:, 0:MT * BL], xv[:, 0, :, :, dt])
                    nc.vector.tensor_add(rz_sb[:, MT * BL:],
                                         prz[:, MT * BL:], xv[:, 1, :, :, dt])
                    rz_act = step_pool.tile([128, 2 * MT * BL], F32, tag="rz_act")
                    nc.scalar.activation(rz_act, rz_sb, ActF.Sigmoid)
                    rh = step_pool.tile([128, MT * BL], F16, tag="rh")
                    nc.vector.tensor_mul(rh, rz_act[:, 0:MT * BL], hf[p])
                    pn_ps = pn_pool.tile([128, MT * BL], F32, tag="pn")
                    for m in range(MT):
                        for k in range(KT):
                            nc.tensor.matmul(
                                pn_ps[:, m * BL:(m + 1) * BL],
                                whT[k][:, 2 * H + m * 128: 2 * H + (m + 1) * 128],
                                rh[:, k * BL:(k + 1) * BL],
                                start=(k == 0), stop=(k == KT - 1))
                    n_sb = step_pool.tile([128, MT * BL], F32, tag="n_sb")
                    nc.vector.tensor_add(n_sb, pn_ps, xv[:, 2, :, :, dt])
                    n_act = step_pool.tile([128, MT * BL], F32, tag="n_act")
                    nc.scalar.activation(n_act, n_sb, ActF.Tanh)
                    d_sb = step_pool.tile([128, MT * BL], F32, tag="d_sb")
                    nc.vector.tensor_sub(d_sb, n_act, hf[p])
                    v_sb = step_pool.tile([128, MT * BL], F32, tag="v_sb")
                    nc.vector.tensor_mul(v_sb, rz_act[:, MT * BL:], d_sb)
                    nc.vector.tensor_add(hf[pn], hf[p], v_sb)
                    nc.scalar.copy(h16[pn], hf[pn])
                    nc.scalar.copy(ov[:, :, :, dt], hf[pn])

                # ---- transpose chunk outputs to [token, H] and store ----
                for q in range(ntt):
                    b0 = q * (128 // tc_sz)
                    st = stage_pool.tile([128, H], F32, tag="stage")
                    for m in range(MT):
                        tp16 = tp_pool.tile([128, 128], F16, tag="tp")
                        nc.tensor.transpose(
                            tp16, out_buf[:, m * ntok + q * 128: m * ntok + (q + 1) * 128],
                            ident16)
                        nc.vector.tensor_copy(st[:, m * 128:(m + 1) * 128], tp16)
                    for bb in range(128 // tc_sz):
                        nc.sync.dma_start(y_d[b0 + bb, t0:t0 + tc_sz, :],
                                          st[bb * tc_sz:(bb + 1) * tc_sz, :])

    nc.compile()
    return nc


_CACHE = {}


def _get_nc():
    if "nc" not in _CACHE:
        _CACHE["nc"] = build()
    return _CACHE["nc"]


def kernel(x, h_0, W_ir, W_iz, W_in, W_hr, W_hz, W_hn, b_hr, b_hz, b_hn):
    args = dict(x=x, h_0=h_0, W_ir=W_ir, W_iz=W_iz, W_in=W_in, W_hr=W_hr,
                W_hz=W_hz, W_hn=W_hn, b_hr=b_hr, b_hz=b_hz, b_hn=b_hn)
    args = {k: np.ascontiguousarray(np.asarray(v, dtype=np.float32))
            for k, v in args.items()}
    nc = _get_nc()
    in_maps = []
    for c in range(NCORES):
        m = {n: args[n] for n in W_NAMES + B_NAMES}
        m["x"] = args["x"][c * BL:(c + 1) * BL]
        m["h_0"] = args["h_0"][c * BL:(c + 1) * BL]
        in_maps.append(m)
    res = run_bass_kernel_spmd(nc, in_maps, core_ids=list(range(NCORES)))
    outputs = np.concatenate([res.results[c]["y"] for c in range(NCORES)],
                             axis=0)
    return outputs, outputs[:, -1].copy()


# revision 16
# speedup vs baseline: 1.1386x; 1.1269x over previous
"""GRU layer kernel for Trainium2 (8 NeuronCores, data-parallel over batch).

Problem: B=64, T=1024, I=H=512 GRU (cuDNN-style gates with z roles swapped):
    r = sigmoid(x W_ir^T + h W_hr^T + b_hr)
    z = sigmoid(x W_iz^T + h W_hz^T + b_hz)
    n = tanh  (x W_in^T + (r*h) W_hn^T + b_hn)
    h' = (1-z)*h + z*n

Design (per core, B_loc=8):
  * Everything lives in [feature-on-partitions, batch-on-free] layout.
  * Input projections are hoisted: per time-chunk of 64 steps, 3 big GEMMs
    compute pre-activations xr/xz/xn for 512 tokens at once (fp16 operands,
    fp32 PSUM accumulate), biases folded in during the PSUM->SBUF copy.
  * The sequential recurrence runs 48 small matmuls per step (3 gates x
    4 K-tiles x 4 M-tiles, stationary fp16 weights, 8-column moving h).
    Measured cost is LDWEIGHTS-bound (~105ns per 128x128 stationary tile);
    the per-step elementwise chain hides entirely under the weight loads
    via the PE's load-ahead window. h is kept as an fp32 master copy
    (elementwise math) plus an fp16 streaming copy.
  * Outputs accumulate per chunk in fp16 [H, token] layout, are
    PE-transposed (single-pass fp16) to [token, H] and DMA'd out densely.
"""

import numpy as np

import concourse.bass as bass
import concourse.mybir as mybir
import concourse.tile as tile
from concourse import bacc
from concourse.bass_utils import run_bass_kernel_spmd
from concourse.masks import make_identity

B, T, I, H = 64, 1024, 512, 512
NCORES = 8
BL = B // NCORES  # 8 batch elements per core
TC = 64           # time steps per chunk
KT = I // 128     # 4 contraction tiles
MT = H // 128     # 4 output tiles per gate
GT = 3            # gates r, z, n
F32 = mybir.dt.float32
F16 = mybir.dt.float16

W_NAMES = ["W_ir", "W_iz", "W_in", "W_hr", "W_hz", "W_hn"]
B_NAMES = ["b_hr", "b_hz", "b_hn"]

ActF = mybir.ActivationFunctionType


def build(nc_T=T, tc_sz=TC):
    nchunk = nc_T // tc_sz
    ntok = BL * tc_sz        # tokens per chunk (512 at TC=64)
    ntt = ntok // 128        # 128-token tiles per chunk

    nc = bacc.Bacc("TRN2", target_bir_lowering=False, debug=False,
                   num_devices=NCORES)
    x_d = nc.dram_tensor("x", [BL, nc_T, I], F32, kind="ExternalInput").ap()
    h0_d = nc.dram_tensor("h_0", [BL, H], F32, kind="ExternalInput").ap()
    w_d = {n: nc.dram_tensor(n, [H, I], F32, kind="ExternalInput").ap()
           for n in W_NAMES}
    b_d = {n: nc.dram_tensor(n, [H], F32, kind="ExternalInput").ap()
           for n in B_NAMES}
    y_d = nc.dram_tensor("y", [BL, nc_T, H], F32, kind="ExternalOutput").ap()

    with tile.TileContext(nc) as tcx, \
            tcx.tile_pool(name="consts", bufs=1) as consts:
        ident = consts.tile([128, 128], F32, name="ident")
        make_identity(nc, ident)
        ident16 = consts.tile([128, 128], F16, name="ident16")
        nc.vector.tensor_copy(ident16, ident)

        # Transposed fp16 weights, one tile per K-tile, gates concatenated in
        # the free dim: wiT[k][:, g*H + j] = W_i<g>[j, k*128 + p].
        wiT = [consts.tile([128, GT * H], F16, name=f"wiT{k}") for k in range(KT)]
        whT = [consts.tile([128, GT * H], F16, name=f"whT{k}") for k in range(KT)]
        b_sb = consts.tile([128, GT * MT], F32, name="b_sb")
        for g, n in enumerate(B_NAMES):
            nc.sync.dma_start(b_sb[:, g * MT:(g + 1) * MT],
                              b_d[n].rearrange("(m p) -> p m", p=128))

        with (
            tcx.tile_pool(name="wraw", bufs=2) as wraw_pool,
            tcx.tile_pool(name="tp_ps", bufs=1, space="PSUM") as tp_pool,
            tcx.tile_pool(name="gemm_ps", bufs=2, space="PSUM") as gemm_pool,
            tcx.tile_pool(name="prz_ps", bufs=2, space="PSUM") as prz_pool,
            tcx.tile_pool(name="pn_ps", bufs=1, space="PSUM") as pn_pool,
            tcx.tile_pool(name="xsb", bufs=3) as xsb_pool,
            tcx.tile_pool(name="xt16", bufs=2) as xt_pool,
            tcx.tile_pool(name="xrt", bufs=2) as xrt_pool,
            tcx.tile_pool(name="outb", bufs=2) as outb_pool,
            tcx.tile_pool(name="stage", bufs=3) as stage_pool,
            tcx.tile_pool(name="step", bufs=3) as step_pool,
        ):
            # ---- one-time: transpose weights into [i, j] fp16 layout ----
            for wi, n in enumerate(W_NAMES):
                dest = wiT if wi < 3 else whT
                g = wi % 3
                for jt in range(4):
                    wr = wraw_pool.tile([128, I], F32, tag="wraw")
                    nc.sync.dma_start(wr, w_d[n][jt * 128:(jt + 1) * 128, :])
                    wr16 = wraw_pool.tile([128, I], F16, tag="wraw16")
                    nc.vector.tensor_copy(wr16, wr)
                    for it in range(KT):
                        tp = tp_pool.tile([128, 128], F16, tag="tp")
                        nc.tensor.transpose(tp, wr16[:, it * 128:(it + 1) * 128],
                                            ident16)
                        nc.vector.tensor_copy(
                            dest[it][:, g * H + jt * 128: g * H + (jt + 1) * 128],
                            tp)

            # ---- one-time: h0 -> transposed fp32 + fp16 ping-pong bufs ----
            hf = [consts.tile([128, MT * BL], F32, name=f"hf{i}") for i in range(2)]
            h16 = [consts.tile([128, MT * BL], F16, name=f"h16_{i}") for i in range(2)]
            h0_sb = consts.tile([BL, H], F32, name="h0_sb")
            nc.sync.dma_start(h0_sb, h0_d)
            for k in range(KT):
                tp = tp_pool.tile([128, BL], F32, tag="tp")
                nc.tensor.transpose(tp, h0_sb[:, k * 128:(k + 1) * 128],
                                    ident[0:BL, 0:BL])
                nc.vector.tensor_copy(hf[0][:, k * BL:(k + 1) * BL], tp)
            nc.scalar.copy(h16[0], hf[0])

            for c in range(nchunk):
                t0 = c * tc_sz
                # ---- input projections for this chunk ----
                xt = xt_pool.tile([128, KT * ntok], F16, tag="xt")
                for q in range(ntt):
                    b0 = q * (128 // tc_sz)
                    xs = xsb_pool.tile([128, I], F32, tag="xs")
                    for bb in range(128 // tc_sz):
                        nc.sync.dma_start(xs[bb * tc_sz:(bb + 1) * tc_sz, :],
                                          x_d[b0 + bb, t0:t0 + tc_sz, :])
                    xs16 = xsb_pool.tile([128, I], F16, tag="xs16")
                    nc.vector.tensor_copy(xs16, xs)
                    for k in range(KT):
                        tp16 = tp_pool.tile([128, 128], F16, tag="tp")
                        nc.tensor.transpose(tp16, xs16[:, k * 128:(k + 1) * 128],
                                            ident16)
                        nc.vector.tensor_copy(
                            xt[:, k * ntok + q * 128: k * ntok + (q + 1) * 128],
                            tp16)
                xrt = xrt_pool.tile([128, GT * MT * ntok], F32, tag="xrt")
                for g in range(GT):
                    for m in range(MT):
                        gp = gemm_pool.tile([128, ntok], F32, tag="gemm")
                        for k in range(KT):
                            nc.tensor.matmul(
                                gp,
                                wiT[k][:, g * H + m * 128: g * H + (m + 1) * 128],
                                xt[:, k * ntok:(k + 1) * ntok],
                                start=(k == 0), stop=(k == KT - 1))
                        nc.scalar.add(
                            xrt[:, (g * MT + m) * ntok:(g * MT + m + 1) * ntok],
                            gp, add=b_sb[:, g * MT + m: g * MT + m + 1])

                # chunk pre-acts viewed as [p, gate, mtile, batch, dt]
                xv = xrt.rearrange("p (g m b t) -> p g m b t", g=GT, m=MT, b=BL)
                out_buf = outb_pool.tile([128, MT * ntok], F16, tag="outb")
                ov = out_buf.rearrange("p (m b t) -> p m b t", m=MT, b=BL)

                # ---- sequential recurrence over the chunk ----
                for dt in range(tc_sz):
                    t = t0 + dt
                    p, pn = t % 2, (t + 1) % 2
                    p_r = prz_pool.tile([128, MT * BL], F32, tag="pr")
                    p_z = prz_pool.tile([128, MT * BL], F32, tag="pz")
                    for g, ps in ((0, p_r), (1, p_z)):
                        for m in range(MT):
                            o0 = m * BL
                            for k in range(KT):
                                nc.tensor.matmul(
                                    ps[:, o0:o0 + BL],
                                    whT[k][:, g * H + m * 128: g * H + (m + 1) * 128],
                                    h16[p][:, k * BL:(k + 1) * BL],
                                    start=(k == 0), stop=(k == KT - 1))
                    rz_sb = step_pool.tile([128, 2 * MT * BL], F32, tag="rz_sb")
                    rz_act = step_pool.tile([128, 2 * MT * BL], F32, tag="rz_act")
                    nc.vector.tensor_add(rz_sb[:, 0:MT * BL],
                                         p_r, xv[:, 0, :, :, dt])
                    nc.scalar.activation(rz_act[:, 0:MT * BL],
                                         rz_sb[:, 0:MT * BL], ActF.Sigmoid)
                    rh = step_pool.tile([128, MT * BL], F16, tag="rh")
                    nc.vector.tensor_mul(rh, rz_act[:, 0:MT * BL], hf[p])
                    nc.vector.tensor_add(rz_sb[:, MT * BL:],
                                         p_z, xv[:, 1, :, :, dt])
                    nc.scalar.activation(rz_act[:, MT * BL:],
                                         rz_sb[:, MT * BL:], ActF.Sigmoid)
                    pn_ps = pn_pool.tile([128, MT * BL], F32, tag="pn")
                    for m in range(MT):
                        for k in range(KT):
                            nc.tensor.matmul(
                                pn_ps[:, m * BL:(m + 1) * BL],
                                whT[k][:, 2 * H + m * 128: 2 * H + (m + 1) * 128],
                                rh[:, k * BL:(k + 1) * BL],
                                start=(k == 0), stop=(k == KT - 1))
                    n_sb = step_pool.tile([128, MT * BL], F32, tag="n_sb")
                    nc.vector.tensor_add(n_sb, pn_ps, xv[:, 2, :, :, dt])
                    n_act = step_pool.tile([128, MT * BL], F32, tag="n_act")
                    nc.scalar.activation(n_act, n_sb, ActF.Tanh)
                    d_sb = step_pool.tile([128, MT * BL], F32, tag="d_sb")
                    nc.vector.tensor_sub(d_sb, n_act, hf[p])
                    v_sb = step_pool.tile([128, MT * BL], F32, tag="v_sb")
                    nc.vector.tensor_mul(v_sb, rz_act[:, MT * BL:], d_sb)
                    nc.vector.tensor_add(hf[pn], hf[p], v_sb)
                    nc.scalar.copy(h16[pn], hf[pn])
                    nc.scalar.copy(ov[:, :, :, dt], hf[pn])

                # ---- transpose chunk outputs to [token, H] and store ----
                for q in range(ntt):
                    b0 = q * (128 // tc_sz)
                    st = stage_pool.tile([128, H], F32, tag="stage")
                    for m in range(MT):
                        tp16 = tp_pool.tile([128, 128], F16, tag="tp")
                        nc.tensor.transpose(
                            tp16, out_buf[:, m * ntok + q * 128: m * ntok + (q + 1) * 128],
                            ident16)
                        nc.vector.tensor_copy(st[:, m * 128:(m + 1) * 128], tp16)
                    for bb in range(128 // tc_sz):
                        nc.sync.dma_start(y_d[b0 + bb, t0:t0 + tc_sz, :],
                                          st[bb * tc_sz:(bb + 1) * tc_sz, :])

    nc.compile()
    return nc


_CACHE = {}


def _get_nc():
    if "nc" not in _CACHE:
        _CACHE["nc"] = build()
    return _CACHE["nc"]


def kernel(x, h_0, W_ir, W_iz, W_in, W_hr, W_hz, W_hn, b_hr, b_hz, b_hn):
    args = dict(x=x, h_0=h_0, W_ir=W_ir, W_iz=W_iz, W_in=W_in, W_hr=W_hr,
                W_hz=W_hz, W_hn=W_hn, b_hr=b_hr, b_hz=b_hz, b_hn=b_hn)
    args = {k: np.ascontiguousarray(np.asarray(v, dtype=np.float32))
            for k, v in args.items()}
    nc = _get_nc()
    in_maps = []
    for c in range(NCORES):
        m = {n: args[n] for n in W_NAMES + B_NAMES}
        m["x"] = args["x"][c * BL:(c + 1) * BL]
        m["h_0"] = args["h_0"][c * BL:(c + 1) * BL]
        in_maps.append(m)
    res = run_bass_kernel_spmd(nc, in_maps, core_ids=list(range(NCORES)))
    outputs = np.concatenate([res.results[c]["y"] for c in range(NCORES)],
                             axis=0)
    return outputs, outputs[:, -1].copy()
